# revision 4
# baseline (speedup 1.0000x reference)
"""HarmonicOscillator Trainium2 kernel, v3 (PE-centric, batched DMA).

out[n,t] = (1/16)*sum_h exp(amps)_up[n,h,t]*sin(2*pi*(h+1)*Phi(t)),
Phi = cumsum(f0_up/SR).

Per half-segment (480 samples) Phi is an exact quadratic in j, hence so is
each per-harmonic phase (h+1)*Phi. Each [128, 960] tile (32 segment-rows x 4
harmonics) synthesizes its phases with one fp32r matmul per half against a
constant integer-valued basis: 32 blocks of 15 samples x {one-hot, j_loc}
plus one global (j-240)^2 row (65 contraction rows). Integer basis values
are exact in bf16 so fp32r error is only the ~2^-16 coefficient split. The
host (fp64) wraps each block's constant so |phase| <= 9.01 rad < 3*pi; one
DVE ADD_RANGE_WRAP per tile folds into [-pi, pi]; ACT evaluates Sin -> fp16;
an fp16 matmul per half contracts each tile's 4 harmonics against amp line
coefficients {c0, c1}; the 4 tiles of a quad (same 32 seg-rows, harmonic
subsets) accumulate into a dense [A0(32); A1(32)] PSUM slab at partition 0
or 64. ACT copies each full 2-bank octet to SBUF, one DMA per octet ships
it, and the host finishes out = A0 + (j/512)*A1.

DMAs are batched 8 tiles per transfer with host-packed contiguous rows so
the HWDGE fixed overhead (625 ns serialized per DMA) stays small: 25 DMAs
total per core.

Sharding: data-parallel over batch N=16 across 8 cores (2 samples/core).
"""
import sys, math, os
sys.path.insert(0, '/opt/trn_rl_repo')
import numpy as np

N, NH, LF = 16, 16, 256
SEG, HSEG = 960, 480
SR = 48000.0
LW = LF * SEG
NCORES = 8
SPC = N // NCORES            # samples per core
ROWS = SPC * LF              # 512 seg-rows per core
P = 128
TIL = ROWS // 8              # 64 tiles; tile = 32 seg-rows x 4 harmonics
NOCT = TIL // 8              # 8 octets (2 quads of 4 tiles -> 1 psum pair)
NBAT = TIL // 8              # 8 input batches of 8 tiles
NB, BS = 32, 15              # blocks per half, block size
K1 = 2 * NB + 1              # 65 contraction rows for the phase matmul
TWO_PI = 2.0 * math.pi
C1SCALE = 512.0              # keep fp16 amp-slope coeffs out of subnormals

_KERNEL_CACHE = {}
_SHRINK = set(os.environ.get("K2_SHRINK", "").split(","))


def _build_nc():
    from concourse import bass, mybir

    def fr(ap, eng):
        """Shrink an op's free dim to 8 for engine-load bisection."""
        return ap[:, 0:8] if eng in _SHRINK else ap

    F32 = mybir.dt.float32
    F32R = mybir.dt.float32r
    F16 = mybir.dt.float16
    Act = mybir.ActivationFunctionType
    Alu = mybir.AluOpType
    PI = float(np.float32(math.pi))
    M_RND = 12582912.0       # 1.5*2^23: (x+M)-M == round(x) for |x| < 2^22

    nc = bass.Bass("TRN2", target_bir_lowering=False, debug=False)

    # host-packed batches: l1 row k holds 8 tiles x 256 cols contiguously
    l1_ext = nc.dram_tensor("l1", [NBAT * K1, 8 * 256], F32R,
                            kind="ExternalInput")
    l2_ext = nc.dram_tensor("l2", [NBAT * P, 8 * 128], F16,
                            kind="ExternalInput")
    b1_ext = nc.dram_tensor("b1", [K1, HSEG], F32R, kind="ExternalInput")
    o2_ext = nc.dram_tensor("o2", [NOCT * P, SEG], F16,
                            kind="ExternalOutput")

    def sb(name, shape, dtype=F32):
        return nc.alloc_sbuf_tensor(name, shape, dtype).ap()

    B1s = sb("B1s", [K1, HSEG], F32R)
    L1B = [sb(f"L1B_{b}", [K1, 8 * 256], F32R) for b in range(2)]
    L2B = [sb(f"L2B_{b}", [P, 8 * 128], F16) for b in range(2)]
    R = [sb(f"R{b}", [P, SEG], F32) for b in range(4)]
    S = [sb(f"S{b}", [P, SEG], F16) for b in range(4)]
    CS = [sb(f"CS{gp}", [P, SEG], F16) for gp in range(2)]
    U = sb("U", [P, HSEG], F32)

    # phase psum: 3-deep [128, 1024] (2 banks each); halves at cols 0 / 512
    PH = [nc.alloc_psum_tensor(f"PH{b}", [P, 1024], F32).ap()
          for b in range(3)]
    C = nc.alloc_psum_tensor("C", [P, 1024], F32).ap()

    def ph2(ap):
        """[128, 960] two-chunk free AP over a [128, 1024] psum tensor."""
        return bass.AP(ap.tensor, ap.offset, [[1024, P], [512, 2], [1, HSEG]])

    # ---- stream op orders (pass 1: assign per-engine indices) ------------
    pe_ops = [("m1", 0, 0), ("m1", 0, 1), ("m1", 1, 0), ("m1", 1, 1),
              ("m1", 2, 0), ("m1", 2, 1)]
    for t in range(TIL):
        if t + 3 < TIL:
            pe_ops += [("m1", t + 3, 0), ("m1", t + 3, 1)]
        pe_ops += [("m2", t, 0), ("m2", t, 1)]
    act_ops = []
    for t in range(TIL):
        act_ops.append(("sin", t))
        for o in range(NOCT):
            if min(8 * o + 9, TIL - 1) == t:
                act_ops.append(("copy", o))
    peidx = {op: i + 1 for i, op in enumerate(pe_ops)}
    actidx = {}
    n = 0
    for op in act_ops:
        n += 1 if op[0] == "sin" else 2
        actidx[op] = n            # value when the op (both halves) completes

    with (
        nc.Block() as block,
        nc.semaphore("din") as din,
        nc.semaphore("dout") as dout,
        nc.semaphore("pe_s") as pe_s,
        nc.semaphore("act_s") as act_s,
        nc.semaphore("ve_s") as ve_s,
    ):
        sems = {"din": din, "dout": dout, "pe": pe_s,
                "act": act_s, "ve": ve_s}
        waited = {}

        def wait(eng, ename, sname, val):
            if val <= 0:
                return
            key = (ename, sname)
            if waited.get(key, -1) >= val:
                return
            waited[key] = val
            eng.wait_ge(sems[sname], val)

        # ---- SP: all DMAs -------------------------------------------------
        @block.sync
        def _(sync):
            def indma(b):
                sync.dma_start(
                    out=L1B[b % 2], in_=l1_ext.ap()[b * K1:(b + 1) * K1, :]
                ).then_inc(din, 16)
                sync.dma_start(
                    out=L2B[b % 2], in_=l2_ext.ap()[b * P:(b + 1) * P, :]
                ).then_inc(din, 16)

            def outdma(oct_):
                wait(sync, "sp", "act", actidx[("copy", oct_)])
                sync.dma_start(
                    out=o2_ext.ap()[P * oct_:P * (oct_ + 1), :],
                    in_=CS[oct_ % 2],
                ).then_inc(dout, 16)

            sync.dma_start(out=B1s, in_=b1_ext.ap()).then_inc(din, 16)
            indma(0)
            sync.wait_ge(din, 48)      # boundary: batch 0 fully landed
            indma(1)
            for b in range(2, NBAT):
                wait(sync, "sp", "pe", peidx[("m2", 8 * b - 9, 1)])
                indma(b)
                outdma(b - 2)
            outdma(6)
            outdma(7)
            sync.wait_ge(dout, 16 * NOCT)

        # ---- PE: phase matmul (m1) + harmonic contraction (m2) -----------
        @block.tensor
        def _(tensor):
            def din_val(b):
                # cumulative DMA totals at SP issue-group boundaries
                return 48 if b == 0 else (80 if b == 1 else 32 * b + 48)

            def m1(t, x):
                b = t // 8
                u = t % 8
                wait(tensor, "pe", "din", din_val(b))
                if t >= 3:
                    # PH[t%3] WAR vs wrap of t-3 (both halves, 4 ve ops/tile)
                    wait(tensor, "pe", "ve", 4 * (t - 3) + 4)
                tensor.matmul(
                    fr(PH[t % 3][:, 512 * x:512 * x + HSEG], "pe"),
                    L1B[b % 2][:, 256 * u + 128 * x:256 * u + 128 * (x + 1)],
                    fr(B1s, "pe"),
                    start=True, stop=True,
                ).then_inc(pe_s)

            def m2(t, x):
                b = t // 8
                u = t % 8
                oct_ = t // 8
                v = (t // 4) % 2
                k = t % 4
                wait(tensor, "pe", "din", din_val(b))
                wait(tensor, "pe", "act", actidx[("sin", t)])
                if oct_ >= 1:
                    wait(tensor, "pe", "act", actidx[("copy", oct_ - 1)])
                tensor.matmul(
                    fr(C[64 * v:64 * (v + 1), 512 * x:512 * x + HSEG], "pe"),
                    L2B[b % 2][:, 128 * u + 64 * x:128 * u + 64 * (x + 1)],
                    fr(S[t % 4][:, HSEG * x:HSEG * (x + 1)], "pe"),
                    start=(k == 0), stop=(k == 3),
                ).then_inc(pe_s)

            for op in pe_ops:
                if op[0] == "m1":
                    m1(op[1], op[2])
                else:
                    m2(op[1], op[2])

        # ---- DVE: range wrap into [-pi, pi], one op per tile -------------
        @block.vector
        def _(vector):
            for t in range(TIL):
                for x in range(2):
                    wait(vector, "ve", "pe", peidx[("m1", t, x)])
                    if t >= 4:
                        wait(vector, "ve", "act", actidx[("sin", t - 4)])
                    phs = fr(PH[t % 3][:, 512 * x:512 * x + HSEG], "ve")
                    vector.tensor_scalar(
                        fr(U, "ve"), phs, M_RND, M_RND,
                        Alu.add, Alu.subtract,
                    ).then_inc(ve_s)
                    vector.tensor_tensor(
                        fr(R[t % 4][:, HSEG * x:HSEG * (x + 1)], "ve"),
                        phs, fr(U, "ve"), Alu.subtract,
                    ).then_inc(ve_s)

        # ---- ACT: sin (1/tile) + psum->sbuf octet copies -----------------
        @block.scalar
        def _(scalar):
            def sin(t):
                wait(scalar, "act", "ve", 4 * t + 4)
                if t >= 4:
                    wait(scalar, "act", "pe", peidx[("m2", t - 4, 1)])
                scalar.activation(
                    fr(S[t % 4], "act"), fr(R[t % 4], "act"),
                    Act.Sin, scale=float(TWO_PI),
                ).then_inc(act_s)

            def copy(o):
                wait(scalar, "act", "pe", peidx[("m2", 8 * o + 7, 1)])
                if o >= 2:
                    wait(scalar, "act", "dout", 16 * o)
                for x in range(2):
                    scalar.activation(
                        fr(CS[o % 2][:, HSEG * x:HSEG * (x + 1)], "act"),
                        fr(C[:, 512 * x:512 * x + HSEG], "act"),
                        Act.Copy,
                    ).then_inc(act_s)

            for op in act_ops:
                if op[0] == "sin":
                    sin(op[1])
                else:
                    copy(op[1])

    return nc


def _host_precompute(amps, f0):
    """fp64 host-side: batched phase-basis coeffs (l1), amp coeffs (l2),
    and the constant basis (b1).

    Tile t = 4*Q + k covers seg-rows 32Q..32Q+31 (rr = n_local*LF + s) and
    harmonics 4k..4k+3; partition p = h_local*32 + r. Batch b = tiles
    8b..8b+7, packed so each DRAM row is contiguous across the batch."""
    f0c = np.maximum(f0[:, 0, :].astype(np.float64), 20.0)        # [N, LF]
    t = np.arange(LW, dtype=np.float64)
    pos = np.clip((t + 0.5) / SEG - 0.5, 0.0, LF - 1)
    i0 = np.floor(pos).astype(np.int64)
    i1 = np.minimum(i0 + 1, LF - 1)
    wfrac = pos - i0
    f0_up = f0c[:, i0] * (1.0 - wfrac) + f0c[:, i1] * wfrac        # [N, LW]
    dt = np.cumsum(f0_up / SR, axis=1)                             # [N, LW]

    # quadratic Q(j) = a + b j + c j^2 per (n, seg, half), j local 0..479
    d4 = dt.reshape(N, LF, 2, HSEG)
    ph0, ph1, ph2_ = d4[..., 0], d4[..., 1], d4[..., 2]
    qc = (ph2_ - 2.0 * ph1 + ph0) * 0.5                            # [N,LF,2]
    qb = (ph1 - ph0) - qc
    qa = ph0
    j479 = qa + qb * 479.0 + qc * 479.0 * 479.0
    assert np.abs(j479 - d4[..., 479]).max() < 1e-6, "phase not quadratic"

    hmul = np.arange(1, NH + 1, dtype=np.float64)                  # [NH]
    jc = (BS * np.arange(NB) + 7).astype(np.float64)               # [NB]
    delta = jc - 240.0                                             # [NB]
    # per (n,s,x,h,b): T = (h+1)Q(jc); Bc = (h+1)Q'(jc); quad = (h+1)qc
    Qjc = qa[..., None] + qb[..., None] * jc + qc[..., None] * jc * jc
    Qp = qb[..., None] + 2.0 * qc[..., None] * jc                  # [N,LF,2,NB]
    T = hmul[None, None, None, :, None] * Qjc[:, :, :, None, :]    # [N,LF,2,NH,NB]
    Bc = hmul[None, None, None, :, None] * Qp[:, :, :, None, :]
    quad = hmul[None, None, None, :, None] * qc[:, :, :, None, None]
    # global (j-240)^2 row absorbs the curvature:
    # phase = A' + B'*jb + quad*(j-240)^2, A' = frac_c(T) - quad*delta^2,
    # B' = Bc - 2*quad*delta
    Ap = (T - np.round(T)) - quad * (delta * delta)
    Bp = Bc - 2.0 * quad * delta

    ampv = np.exp(amps.astype(np.float64)) / NH                    # [N,NH,LF]
    am = np.concatenate([ampv[:, :, 0:1], ampv[:, :, :-1]], axis=2)
    dv = ampv - am
    c0h0 = am + dv * (480.5 / SEG)
    c1h0 = dv / SEG * C1SCALE
    an = np.concatenate([ampv[:, :, 1:], ampv[:, :, -1:]], axis=2)
    ev = an - ampv
    c0h1 = ampv + ev * (0.5 / SEG)
    c1h1 = ev / SEG * C1SCALE

    l1 = np.zeros((NCORES, TIL, K1, 256), dtype=np.float64)
    l2 = np.zeros((NCORES, TIL, P, 128), dtype=np.float64)

    def to_tiles(arr):
        """[SPC, LF, 2, NH, NB] -> [2, TIL, NB, 128] (x, tile, block, p)."""
        a = arr.reshape(ROWS, 2, NH, NB).reshape(16, 32, 2, 4, 4, NB)
        a = a.transpose(2, 0, 3, 5, 4, 1)           # [x, Q, k, NB, hl, r]
        return a.reshape(2, TIL, NB, 128)

    def to_tiles1(arr):
        """[SPC, LF, 2, NH] -> [2, TIL, 128]."""
        a = arr.reshape(ROWS, 2, NH).reshape(16, 32, 2, 4, 4)
        a = a.transpose(2, 0, 3, 4, 1)              # [x, Q, k, hl, r]
        return a.reshape(2, TIL, 128)

    for core in range(NCORES):
        ns = [2 * core, 2 * core + 1]
        at = to_tiles(Ap[ns])
        bt = to_tiles(Bp[ns])
        qt = to_tiles1(np.broadcast_to(
            quad[ns][..., 0], (SPC, LF, 2, NH)))
        for x in range(2):
            col = 128 * x
            l1[core, :, 0:NB, col:col + 128] = at[x]
            l1[core, :, NB:2 * NB, col:col + 128] = bt[x]
            l1[core, :, 2 * NB, col:col + 128] = qt[x]
        for x, (c0s, c1s) in enumerate(((c0h0, c1h0), (c0h1, c1h1))):
            # [SPC, NH, LF] -> [Q, r, k, hl]
            c0r = c0s[ns].transpose(0, 2, 1).reshape(16, 32, 4, 4)
            c1r = c1s[ns].transpose(0, 2, 1).reshape(16, 32, 4, 4)
            for k in range(4):
                for hl in range(4):
                    pbase = hl * 32
                    rows = np.arange(32)
                    l2[core, k::4, pbase + rows, 64 * x + rows] = \
                        c0r[:, :, k, hl].T
                    l2[core, k::4, pbase + rows, 64 * x + 32 + rows] = \
                        c1r[:, :, k, hl].T

    # batch packing: row-contiguous across 8 tiles
    l1b = l1.reshape(NCORES, NBAT, 8, K1, 256).transpose(0, 1, 3, 2, 4)
    l1b = l1b.reshape(NCORES, NBAT * K1, 8 * 256)
    l2b = l2.reshape(NCORES, NBAT, 8, P, 128).transpose(0, 1, 3, 2, 4)
    l2b = l2b.reshape(NCORES, NBAT * P, 8 * 128)

    # constant basis [K1, HSEG]: integer-valued, exact in bf16
    jj = np.arange(HSEG, dtype=np.float64)
    blk = (jj // BS).astype(np.int64)
    jloc = jj - (BS * blk + 7)
    b1 = np.zeros((K1, HSEG), dtype=np.float64)
    for b in range(NB):
        m = blk == b
        b1[b, m] = 1.0
        b1[NB + b, m] = jloc[m]
    b1[2 * NB] = (jj - 240.0) ** 2

    return (l1b.astype(np.float32), l2b.astype(np.float16),
            b1.astype(np.float32))


def _postprocess(o2):
    """o2 [1024, 960] per core -> [SPC, 1, LW]. Row 128*oct + 64v + c:
    quad Q = 2*oct + v covers seg-rows 32Q..32Q+31; c<32 => A0 row c,
    c>=32 => A1 row c-32."""
    o5 = o2.reshape(NOCT, 2, 2, 32, SEG)           # [oct, v, a, r, j]
    A0 = o5[:, :, 0, :, :].astype(np.float64)
    A1 = o5[:, :, 1, :, :].astype(np.float64)
    jj = np.arange(HSEG, dtype=np.float64) / C1SCALE
    jw = np.concatenate([jj, jj])                  # both halves local j
    res = A0 + A1 * jw                             # [oct, v, r, 960]
    return res.reshape(ROWS, SEG).reshape(SPC, 1, LW).astype(np.float32)


def kernel(amps, f0):
    from concourse.bass_utils import run_bass_kernel_spmd

    if "nc" not in _KERNEL_CACHE:
        _KERNEL_CACHE["nc"] = _build_nc()
    nc = _KERNEL_CACHE["nc"]

    l1b, l2b, b1 = _host_precompute(amps, f0)
    in_maps = []
    for c in range(NCORES):
        in_maps.append({
            "l1": np.ascontiguousarray(l1b[c]),
            "l2": np.ascontiguousarray(l2b[c]),
            "b1": b1,
        })
    res = run_bass_kernel_spmd(nc, in_maps, list(range(NCORES)))
    out = np.concatenate(
        [_postprocess(res.results[c]["o2"]) for c in range(NCORES)], axis=0)
    return out.astype(np.float32)
